# revision 13
# baseline (speedup 1.0000x reference)
"""Trainium2 Bass kernel for nn_AsymmetricContrastiveLoss.

Reference semantics (B=32768, D=2048, TIMEPOINTS=4):
  pos rows = z[labels != 0], neg rows = z[labels == 0]   (equal counts)
  align      = 1 - mean_i cos(zp_i, zp_{perm_i}) + mean_i cos(zp_i, zn_i)
  orthogonal = mean_i (|cos(z0,z1)| + |cos(z1,z2)| + |cos(z2,z3)|) / 3
               where z0..z3 are the 4 chunks (512 wide) of zp_i
  temporal   = mean_i (t1+t2+t3)/3 with t_k = 1 - cos(u_k, v_k) where the
               u_k/v_k pairs are identical telescoping sums of chunk
               differences (u_k == v_k algebraically for any input), so
               each t_k is identically 0 (the reference value is fp32
               round-off noise at ~1e-8).

Sharding: data-parallel over the batch.  The host derives the pos/neg
index sets and the permutation gather order (index-only work, mirroring
the reference's trace-time static partition), and slices three row
streams per core.  Each of the 8 NeuronCores streams its 3x[2048, 2048]
f32 row blocks from HBM once (48 MB/core) and emits per-row sufficient
statistics with fused multiply-reduce ops:

  ScalarE : chunk norms c0..c3 of zp, |zn|^2   (activation Square+accum)
  VectorE : d01,d12,d23 chunk-pair dots, zp.zg and zp.zn row dots
            (scalar_tensor_tensor with accum_out)

The host epilogue (float64, ~100 KB of scalars) forms the cosines and
the three means.  |zp_perm|^2 needs no device work: it is a permutation
of the row norms already computed (identical bytes, identical reduction).
"""

import os

import numpy as np

# Problem constants (hardcoded per task contract).
B = 32768
D = 2048
TD = 512  # chunk width (D / TIMEPOINTS)
N = B // 2  # positive row count
NCORES = 8
R = N // NCORES  # rows per core = 2048
P = 128  # SBUF partitions
T = R // P  # 128-row tiles per core = 16
HS = 2  # tiles fetched per DMA (2 MB loads)
S = T // HS  # DMA steps

_PROGRAM_CACHE = {}


def _build_program():
    import concourse.bacc as bacc
    import concourse.mybir as mybir
    import concourse.tile as tile

    f32 = mybir.dt.float32
    Alu = mybir.AluOpType
    Act = mybir.ActivationFunctionType

    nc = bacc.Bacc("TRN2", target_bir_lowering=False, debug=False,
                   num_devices=NCORES)

    zp = nc.dram_tensor("zp", [R, D], f32, kind="ExternalInput")
    zg = nc.dram_tensor("zg", [R, D], f32, kind="ExternalInput")
    zn = nc.dram_tensor("zn", [R, D], f32, kind="ExternalInput")
    # out_a cols (16 each): c0,c1,c2,c3, nn ; out_v cols: d01,d12,d23, zg, zn
    out_a = nc.dram_tensor("out_a", [P, 5 * 16], f32, kind="ExternalOutput")
    out_v = nc.dram_tensor("out_v", [P, 5 * 16], f32, kind="ExternalOutput")

    with tile.TileContext(nc) as tc:
        with (
            tc.tile_pool(name="io", bufs=3) as io_pool,
            tc.tile_pool(name="scr", bufs=2) as scr_pool,
            tc.tile_pool(name="stats", bufs=1) as st_pool,
        ):
            stats_a = st_pool.tile([P, 5 * 16], f32)
            stats_v = st_pool.tile([P, 5 * 16], f32)

            def col(arr, q, t):
                return arr[:, q * 16 + t:q * 16 + t + 1]

            for s in range(S):
                rows = slice(s * HS * P, (s + 1) * HS * P)
                zpt = io_pool.tile([P, HS * D], f32, tag="zpt")
                zgt = io_pool.tile([P, HS * D], f32, tag="zgt")
                znt = io_pool.tile([P, HS * D], f32, tag="znt")
                for dst, src in ((zpt, zp), (zgt, zg), (znt, zn)):
                    nc.sync.dma_start(
                        out=dst[:].rearrange("p (h d) -> p h d", h=HS),
                        in_=src[rows, :].rearrange("(h p) d -> p h d", p=P))

                act_scr = scr_pool.tile([P, D], f32, tag="act_scr")
                dve_scr = scr_pool.tile([P, D], f32, tag="dve_scr")
                for h in range(HS):
                    t = s * HS + h
                    o = h * D

                    # --- ScalarE: chunk norms of zp, |zn|^2 ---
                    for ci in range(4):
                        cs = slice(o + ci * TD, o + (ci + 1) * TD)
                        nc.scalar.activation(
                            act_scr[:, ci * TD:(ci + 1) * TD], zpt[:, cs],
                            Act.Square, accum_out=col(stats_a, ci, t))
                    nc.scalar.activation(
                        act_scr[:], znt[:, o:o + D], Act.Square,
                        accum_out=col(stats_a, 4, t))

                    # --- VectorE: fused row dots ---
                    for qi, (a, b) in enumerate(((0, 1), (1, 2), (2, 3))):
                        nc.vector.scalar_tensor_tensor(
                            out=dve_scr[:, 0:TD],
                            in0=zpt[:, o + a * TD:o + (a + 1) * TD],
                            scalar=1.0,
                            in1=zpt[:, o + b * TD:o + (b + 1) * TD],
                            op0=Alu.mult, op1=Alu.mult,
                            accum_out=col(stats_v, qi, t))
                    nc.vector.scalar_tensor_tensor(
                        out=dve_scr[:], in0=zpt[:, o:o + D],
                        scalar=1.0, in1=zgt[:, o:o + D],
                        op0=Alu.mult, op1=Alu.mult,
                        accum_out=col(stats_v, 3, t))
                    nc.vector.scalar_tensor_tensor(
                        out=dve_scr[:], in0=zpt[:, o:o + D],
                        scalar=1.0, in1=znt[:, o:o + D],
                        op0=Alu.mult, op1=Alu.mult,
                        accum_out=col(stats_v, 4, t))

            nc.sync.dma_start(out=out_a[:, :], in_=stats_a[:])
            nc.sync.dma_start(out=out_v[:, :], in_=stats_v[:])

    nc.compile()
    return nc


def _get_program():
    if "nc" not in _PROGRAM_CACHE:
        _PROGRAM_CACHE["nc"] = _build_program()
    return _PROGRAM_CACHE["nc"]


def kernel(z, labels, perm):
    from concourse.bass_utils import run_bass_kernel_spmd

    z = np.ascontiguousarray(np.asarray(z), dtype=np.float32)
    labels = np.asarray(labels)
    perm = np.asarray(perm).astype(np.int64)
    assert z.shape == (B, D)

    # Host-side static partition (index-only, mirrors the reference's
    # trace-time np.nonzero) and the per-core row streams.
    lab = np.asarray(labels).astype(bool)
    pos_idx = np.nonzero(lab)[0]
    neg_idx = np.nonzero(~lab)[0]
    assert len(pos_idx) == N and len(neg_idx) == N
    gather_idx = pos_idx[perm]

    in_maps = []
    for c in range(NCORES):
        rows = slice(c * R, (c + 1) * R)
        in_maps.append({
            "zp": np.ascontiguousarray(z[pos_idx[rows]]),
            "zg": np.ascontiguousarray(z[gather_idx[rows]]),
            "zn": np.ascontiguousarray(z[neg_idx[rows]]),
        })

    nc = _get_program()
    res = run_bass_kernel_spmd(nc, in_maps, core_ids=list(range(NCORES)))

    if res.exec_time_ns is not None:
        _PROGRAM_CACHE["exec_time_ns"] = res.exec_time_ns
        _PROGRAM_CACHE["trace"] = res.instructions_and_trace
        if os.environ.get("BASS_TRACE"):
            print(f"HW exec time: {res.exec_time_ns} ns")

    # [NCORES, P, 80] -> per-row arrays indexed by global pos/neg stream row:
    # row (c, t, p) = c*R + t*P + p lives at parts[c, p, q*16+t].
    pa = np.stack([r["out_a"] for r in res.results]).astype(np.float64)
    pv = np.stack([r["out_v"] for r in res.results]).astype(np.float64)

    def rows_of(parts, q):
        # [NCORES, P, 16] -> [NCORES, 16, P] -> [N]
        blk = parts[:, :, q * 16:(q + 1) * 16]
        return blk.transpose(0, 2, 1).reshape(N)

    c0, c1, c2, c3, nn = (rows_of(pa, q) for q in range(5))
    d01, d12, d23, zgd, znd = (rows_of(pv, q) for q in range(5))

    eps = 1e-8
    n2 = c0 + c1 + c2 + c3
    na = np.maximum(np.sqrt(n2), eps)
    nb_neg = np.maximum(np.sqrt(nn), eps)
    na_c = [np.maximum(np.sqrt(c), eps) for c in (c0, c1, c2, c3)]

    cosP = zgd / (na * na[perm])
    cosN = znd / (na * nb_neg)
    orth = (np.abs(d01 / (na_c[0] * na_c[1]))
            + np.abs(d12 / (na_c[1] * na_c[2]))
            + np.abs(d23 / (na_c[2] * na_c[3]))) / 3.0

    return {
        "align": np.float32(1.0 - cosP.mean() + cosN.mean()),
        "orthogonal": np.float32(orth.mean()),
        "temporal": np.float32(0.0),
    }


# revision 14
# speedup vs baseline: 1.1117x; 1.1117x over previous
"""Trainium2 Bass kernel for nn_AsymmetricContrastiveLoss.

Reference semantics (B=32768, D=2048, TIMEPOINTS=4):
  pos rows = z[labels != 0], neg rows = z[labels == 0]   (equal counts)
  align      = 1 - mean_i cos(zp_i, zp_{perm_i}) + mean_i cos(zp_i, zn_i)
  orthogonal = mean_i (|cos(z0,z1)| + |cos(z1,z2)| + |cos(z2,z3)|) / 3
               where z0..z3 are the 4 chunks (512 wide) of zp_i
  temporal   = mean_i (t1+t2+t3)/3 with t_k = 1 - cos(u_k, v_k) where the
               u_k/v_k pairs are identical telescoping sums of chunk
               differences (u_k == v_k algebraically for any input), so
               each t_k is identically 0 (the reference value is fp32
               round-off noise at ~1e-8).

Sharding: data-parallel over the batch.  The host derives the pos/neg
index sets and the permutation gather order (index-only work, mirroring
the reference's trace-time static partition), and slices three row
streams per core.  Each of the 8 NeuronCores streams its 3x[2048, 2048]
f32 row blocks from HBM once (48 MB/core) and emits per-row sufficient
statistics with fused multiply-reduce ops:

  ScalarE : chunk norms c0..c3 of zp, |zn|^2   (activation Square+accum)
  VectorE : d01,d12,d23 chunk-pair dots, zp.zg and zp.zn row dots
            (scalar_tensor_tensor with accum_out)

The host epilogue (float64, ~100 KB of scalars) forms the cosines and
the three means.  |zp_perm|^2 needs no device work: it is a permutation
of the row norms already computed (identical bytes, identical reduction).
"""

import os

import numpy as np

# Problem constants (hardcoded per task contract).
B = 32768
D = 2048
TD = 512  # chunk width (D / TIMEPOINTS)
N = B // 2  # positive row count
NCORES = 8
R = N // NCORES  # rows per core = 2048
P = 128  # SBUF partitions
T = R // P  # 128-row tiles per core = 16
HS = 2  # tiles fetched per DMA (2 MB loads)
S = T // HS  # DMA steps

_PROGRAM_CACHE = {}


def _build_program():
    import concourse.bacc as bacc
    import concourse.mybir as mybir
    import concourse.tile as tile

    f32 = mybir.dt.float32
    Alu = mybir.AluOpType
    Act = mybir.ActivationFunctionType

    nc = bacc.Bacc("TRN2", target_bir_lowering=False, debug=False,
                   num_devices=NCORES)

    zp = nc.dram_tensor("zp", [R, D], f32, kind="ExternalInput")
    zg = nc.dram_tensor("zg", [R, D], f32, kind="ExternalInput")
    zn = nc.dram_tensor("zn", [R, D], f32, kind="ExternalInput")
    # out_a cols (16 each): c0,c1,c2,c3, nn ; out_v cols: d01,d12,d23, zg, zn
    out_a = nc.dram_tensor("out_a", [P, 5 * 16], f32, kind="ExternalOutput")
    out_v = nc.dram_tensor("out_v", [P, 5 * 16], f32, kind="ExternalOutput")

    with tile.TileContext(nc) as tc:
        with (
            tc.tile_pool(name="io", bufs=3) as io_pool,
            tc.tile_pool(name="scr", bufs=2) as scr_pool,
            tc.tile_pool(name="stats", bufs=1) as st_pool,
        ):
            stats_a = st_pool.tile([P, 5 * 16], f32)
            stats_v = st_pool.tile([P, 5 * 16], f32)

            def col(arr, q, t):
                return arr[:, q * 16 + t:q * 16 + t + 1]

            for s in range(S):
                rows = slice(s * HS * P, (s + 1) * HS * P)
                zpt = io_pool.tile([P, HS * D], f32, tag="zpt")
                zgt = io_pool.tile([P, HS * D], f32, tag="zgt")
                znt = io_pool.tile([P, HS * D], f32, tag="znt")
                # Split the three loads across both HWDGE rings (SP + ACT
                # sequencers) so transfer completions overlap.
                zn_eng = nc.sync if s % 2 == 0 else nc.scalar
                for eng, dst, src in ((nc.sync, zpt, zp), (nc.scalar, zgt, zg),
                                      (zn_eng, znt, zn)):
                    eng.dma_start(
                        out=dst[:].rearrange("p (h d) -> p h d", h=HS),
                        in_=src[rows, :].rearrange("(h p) d -> p h d", p=P))

                act_scr = scr_pool.tile([P, D], f32, tag="act_scr")
                dve_scr = scr_pool.tile([P, D], f32, tag="dve_scr")
                for h in range(HS):
                    t = s * HS + h
                    o = h * D

                    # --- ScalarE: chunk norms of zp, |zn|^2 ---
                    for ci in range(4):
                        cs = slice(o + ci * TD, o + (ci + 1) * TD)
                        nc.scalar.activation(
                            act_scr[:, ci * TD:(ci + 1) * TD], zpt[:, cs],
                            Act.Square, accum_out=col(stats_a, ci, t))
                    nc.scalar.activation(
                        act_scr[:], znt[:, o:o + D], Act.Square,
                        accum_out=col(stats_a, 4, t))

                    # --- VectorE: fused row dots ---
                    for qi, (a, b) in enumerate(((0, 1), (1, 2), (2, 3))):
                        nc.vector.scalar_tensor_tensor(
                            out=dve_scr[:, 0:TD],
                            in0=zpt[:, o + a * TD:o + (a + 1) * TD],
                            scalar=1.0,
                            in1=zpt[:, o + b * TD:o + (b + 1) * TD],
                            op0=Alu.mult, op1=Alu.mult,
                            accum_out=col(stats_v, qi, t))
                    nc.vector.scalar_tensor_tensor(
                        out=dve_scr[:], in0=zpt[:, o:o + D],
                        scalar=1.0, in1=zgt[:, o:o + D],
                        op0=Alu.mult, op1=Alu.mult,
                        accum_out=col(stats_v, 3, t))
                    nc.vector.scalar_tensor_tensor(
                        out=dve_scr[:], in0=zpt[:, o:o + D],
                        scalar=1.0, in1=znt[:, o:o + D],
                        op0=Alu.mult, op1=Alu.mult,
                        accum_out=col(stats_v, 4, t))

            nc.sync.dma_start(out=out_a[:, :], in_=stats_a[:])
            nc.sync.dma_start(out=out_v[:, :], in_=stats_v[:])

    nc.compile()
    return nc


def _get_program():
    if "nc" not in _PROGRAM_CACHE:
        _PROGRAM_CACHE["nc"] = _build_program()
    return _PROGRAM_CACHE["nc"]


def kernel(z, labels, perm):
    from concourse.bass_utils import run_bass_kernel_spmd

    z = np.ascontiguousarray(np.asarray(z), dtype=np.float32)
    labels = np.asarray(labels)
    perm = np.asarray(perm).astype(np.int64)
    assert z.shape == (B, D)

    # Host-side static partition (index-only, mirrors the reference's
    # trace-time np.nonzero) and the per-core row streams.
    lab = np.asarray(labels).astype(bool)
    pos_idx = np.nonzero(lab)[0]
    neg_idx = np.nonzero(~lab)[0]
    assert len(pos_idx) == N and len(neg_idx) == N
    gather_idx = pos_idx[perm]

    in_maps = []
    for c in range(NCORES):
        rows = slice(c * R, (c + 1) * R)
        in_maps.append({
            "zp": np.ascontiguousarray(z[pos_idx[rows]]),
            "zg": np.ascontiguousarray(z[gather_idx[rows]]),
            "zn": np.ascontiguousarray(z[neg_idx[rows]]),
        })

    nc = _get_program()
    res = run_bass_kernel_spmd(nc, in_maps, core_ids=list(range(NCORES)))

    if res.exec_time_ns is not None:
        _PROGRAM_CACHE["exec_time_ns"] = res.exec_time_ns
        _PROGRAM_CACHE["trace"] = res.instructions_and_trace
        if os.environ.get("BASS_TRACE"):
            print(f"HW exec time: {res.exec_time_ns} ns")

    # [NCORES, P, 80] -> per-row arrays indexed by global pos/neg stream row:
    # row (c, t, p) = c*R + t*P + p lives at parts[c, p, q*16+t].
    pa = np.stack([r["out_a"] for r in res.results]).astype(np.float64)
    pv = np.stack([r["out_v"] for r in res.results]).astype(np.float64)

    def rows_of(parts, q):
        # [NCORES, P, 16] -> [NCORES, 16, P] -> [N]
        blk = parts[:, :, q * 16:(q + 1) * 16]
        return blk.transpose(0, 2, 1).reshape(N)

    c0, c1, c2, c3, nn = (rows_of(pa, q) for q in range(5))
    d01, d12, d23, zgd, znd = (rows_of(pv, q) for q in range(5))

    eps = 1e-8
    n2 = c0 + c1 + c2 + c3
    na = np.maximum(np.sqrt(n2), eps)
    nb_neg = np.maximum(np.sqrt(nn), eps)
    na_c = [np.maximum(np.sqrt(c), eps) for c in (c0, c1, c2, c3)]

    cosP = zgd / (na * na[perm])
    cosN = znd / (na * nb_neg)
    orth = (np.abs(d01 / (na_c[0] * na_c[1]))
            + np.abs(d12 / (na_c[1] * na_c[2]))
            + np.abs(d23 / (na_c[2] * na_c[3]))) / 3.0

    return {
        "align": np.float32(1.0 - cosP.mean() + cosN.mean()),
        "orthogonal": np.float32(orth.mean()),
        "temporal": np.float32(0.0),
    }
